# revision 11
# baseline (speedup 1.0000x reference)
"""Distributed Trainium2 kernel for GQA attention (B=2, S=2048, D=4096,
32 q-heads / 8 kv-heads, HD=128, RoPE, additive mask) on 8 NeuronCores.

Sharding: tensor-parallel over heads (4 q-heads + 1 kv-head per core).
Each core computes its heads' attention output o^T for ALL tokens, then an
AllToAll exchanges (feature-block, token-slice) pieces so each core holds
all 4096 features for its 512-token slice and runs the full Wo GEMM for
that slice. Host concatenates the 8 token slices. Matmuls run fp32r
(q/k/scores) and bf16 (p@v, Wo).
"""
import sys

sys.path.insert(0, "/opt/trn_rl_repo")

import math
from contextlib import ExitStack
import numpy as np
import ml_dtypes

import concourse.bass as bass
import concourse.tile as tile
from concourse import bacc, mybir
from concourse.bass_utils import run_bass_kernel_spmd
from concourse.masks import make_identity

F32 = mybir.dt.float32
F32R = mybir.dt.float32r
BF16 = mybir.dt.bfloat16
AF = mybir.ActivationFunctionType
AX = mybir.AxisListType
OP = mybir.AluOpType

NCORES = 8
B, S, D = 2, 2048, 4096
NH, NKV, HD = 32, 8, 128
QH = NH // NCORES          # 4 q-heads per core
TOK = B * S                # 4096
TT = TOK // 128            # 32 token tiles
RT = S // 128              # 16 row tiles per batch
TSLICE = TOK // NCORES     # 512 tokens out per core
NEG_INF = -1e9


def _build(mask_mode: str):
    """mask_mode: 'causal' | 'zeros' | 'general'"""
    nc = bacc.Bacc("TRN2", target_bir_lowering=False, debug=False,
                   enable_asserts=True, num_devices=NCORES)

    xT_e = nc.dram_tensor("xT", [D, TOK], F32, kind="ExternalInput")
    Wq_e = nc.dram_tensor("Wq", [D, QH * HD], F32, kind="ExternalInput")
    Wkv_e = nc.dram_tensor("Wkv", [D, 2 * HD], F32, kind="ExternalInput")
    Wo_e = nc.dram_tensor("Wo", [D, D], BF16, kind="ExternalInput")
    cq_e = nc.dram_tensor("cq", [S, 64], F32, kind="ExternalInput")
    sq_e = nc.dram_tensor("sq", [S, 64], F32, kind="ExternalInput")
    ck_e = nc.dram_tensor("ck", [S, 64], F32, kind="ExternalInput")
    sk_e = nc.dram_tensor("sk", [S, 64], F32, kind="ExternalInput")
    if mask_mode == "causal":
        mask_e = nc.dram_tensor("maskd", [S, 512], F32, kind="ExternalInput")
    elif mask_mode == "general":
        mask_e = nc.dram_tensor("mask", [S, S], F32, kind="ExternalInput")
    else:
        mask_e = None
    out_e = nc.dram_tensor("out", [TSLICE, D], F32, kind="ExternalOutput")

    qT_d = nc.dram_tensor("qT_stage", [QH, B, HD, S], F32R)

    with tile.TileContext(nc) as tc, ExitStack() as ctx:
        _body(ctx, tc, mask_mode, xT_e, Wq_e, Wkv_e, Wo_e,
              cq_e, sq_e, ck_e, sk_e, mask_e, out_e, qT_d)
    nc.compile()
    return nc


def _body(ctx, tc, mask_mode, xT_e, Wq_e, Wkv_e, Wo_e,
          cq_e, sq_e, ck_e, sk_e, mask_e, out_e, qT_d):
    nc = tc.nc
    causal = mask_mode == "causal"

    consts = ctx.enter_context(tc.tile_pool(name="consts", bufs=1))
    ident_r = consts.tile([128, 128], F32)
    make_identity(nc, ident_r[:])
    ident_b = consts.tile([128, 128], BF16)
    make_identity(nc, ident_b[:])

    # persistent across phases
    kv_pool = ctx.enter_context(tc.tile_pool(name="kv", bufs=1))
    kT_sb = kv_pool.tile([128, TT, 128], F32R)      # [hd, (b,kt), tok]
    v_sb = kv_pool.tile([128, TT, 128], BF16)       # [tok, (b,kt), feat]

    dram = ctx.enter_context(tc.tile_pool(name="dram", bufs=1, space="DRAM"))
    a2a_in = dram.tile([NCORES, QH * HD, TSLICE], BF16)
    a2a_out = dram.tile([NCORES, QH * HD, TSLICE], BF16)

    # ---------------- Phase 1: projections + RoPE + transposes -----------
    with tc.tile_pool(name="pj_w", bufs=1) as wpool, \
         tc.tile_pool(name="pj_x", bufs=2) as xpool, \
         tc.tile_pool(name="pj_rt", bufs=1) as rtpool, \
         tc.tile_pool(name="pj_tmp", bufs=4) as tmppool, \
         tc.tile_pool(name="pj_ro", bufs=2) as ropool, \
         tc.tile_pool(name="pj_stage", bufs=3) as stpool, \
         tc.tile_pool(name="pj_psq", bufs=2, space="PSUM") as psq_pool, \
         tc.tile_pool(name="pj_pskv", bufs=2, space="PSUM") as pskv_pool, \
         tc.tile_pool(name="pj_pstr", bufs=2, space="PSUM") as pstr_pool:

        Wq_sb = wpool.tile([128, 32, QH * HD], F32R)
        nc.gpsimd.dma_start(out=Wq_sb[:],
                            in_=Wq_e.ap().rearrange("(k p) f -> p k f", p=128))
        Wkv_sb = wpool.tile([128, 32, 2 * HD], F32R)
        nc.gpsimd.dma_start(out=Wkv_sb[:],
                            in_=Wkv_e.ap().rearrange("(k p) f -> p k f", p=128))

        # rope tables resident: [pos%128, pos//128, j]
        ctabs = {}
        for nm, te in (("cq", cq_e), ("sq", sq_e), ("ck", ck_e), ("sk", sk_e)):
            t = rtpool.tile([128, RT, 64], F32, name=f"tab_{nm}", tag=f"tab_{nm}")
            nc.sync.dma_start(out=t[:],
                              in_=te.ap().rearrange("(s p) j -> p s j", p=128))
            ctabs[nm] = t

        def rep4(ap):  # repeat [128,64] table slice 4x along free dim
            return bass.AP(tensor=ap.tensor, offset=ap.offset,
                           ap=[ap.ap[0], [0, QH], ap.ap[-1]])

        for tt in range(TT):
            b, st = tt // RT, tt % RT   # batch, row-tile within batch
            xcol = xpool.tile([128, 32, 128], F32R)
            nc.gpsimd.dma_start(
                out=xcol[:],
                in_=xT_e.ap()[:, tt * 128:(tt + 1) * 128]
                    .rearrange("(k p) t -> p k t", p=128))

            psq = psq_pool.tile([128, QH * HD], F32)
            pskv = pskv_pool.tile([128, 2 * HD], F32)
            for k in range(32):
                nc.tensor.matmul(psq[:], xcol[:, k, :], Wq_sb[:, k, :],
                                 start=(k == 0), stop=(k == 31))
                nc.tensor.matmul(pskv[:], xcol[:, k, :], Wkv_sb[:, k, :],
                                 start=(k == 0), stop=(k == 31))

            # ---- RoPE on q (scaled tables) -> q_ro [tok,feat] f32
            q_ro = ropool.tile([128, QH * HD], F32)
            q3o = q_ro[:].rearrange("p (j two) -> p j two", two=2)
            ps3 = psq[:].rearrange("p (j two) -> p j two", two=2)
            c4 = rep4(ctabs["cq"][:, st, :])
            s4 = rep4(ctabs["sq"][:, st, :])
            t1 = tmppool.tile([128, QH * 64], F32)
            t2 = tmppool.tile([128, QH * 64], F32)
            nc.vector.tensor_mul(t1[:], ps3[:, :, 0], c4)
            nc.vector.tensor_mul(t2[:], ps3[:, :, 1], s4)
            nc.vector.tensor_sub(q3o[:, :, 0], t1[:], t2[:])
            nc.vector.tensor_mul(t1[:], ps3[:, :, 0], s4)
            nc.vector.tensor_mul(t2[:], ps3[:, :, 1], c4)
            nc.vector.tensor_add(q3o[:, :, 1], t1[:], t2[:])

            # ---- RoPE on k (unscaled tables) -> k_ro
            k_ro = ropool.tile([128, HD], F32)
            k3o = k_ro[:].rearrange("p (j two) -> p j two", two=2)
            pk3 = pskv[:, 0:HD].rearrange("p (j two) -> p j two", two=2)
            ck1 = ctabs["ck"][:, st, :]
            sk1 = ctabs["sk"][:, st, :]
            u1 = tmppool.tile([128, 64], F32)
            u2 = tmppool.tile([128, 64], F32)
            nc.vector.tensor_mul(u1[:], pk3[:, :, 0], ck1)
            nc.vector.tensor_mul(u2[:], pk3[:, :, 1], sk1)
            nc.vector.tensor_sub(k3o[:, :, 0], u1[:], u2[:])
            nc.vector.tensor_mul(u1[:], pk3[:, :, 0], sk1)
            nc.vector.tensor_mul(u2[:], pk3[:, :, 1], ck1)
            nc.vector.tensor_add(k3o[:, :, 1], u1[:], u2[:])

            # ---- v: psum -> bf16 resident
            nc.scalar.copy(v_sb[:, tt, :], pskv[:, HD:2 * HD])

            # ---- transpose q -> stage -> DRAM qT; k -> kT resident
            ptq = pstr_pool.tile([128, QH * HD], F32)
            for fh in range(QH):
                nc.tensor.transpose(ptq[:, fh * HD:(fh + 1) * HD],
                                    q_ro[:, fh * HD:(fh + 1) * HD], ident_r[:])
            qst = stpool.tile([128, QH * HD], F32R)
            nc.scalar.copy(qst[:], ptq[:])
            # scatter [hd, (fh tok)] -> qT_d[fh, b, hd, pos]
            dst = bass.AP(
                tensor=qT_d[:].tensor, offset=b * (HD * S) + st * 128,
                ap=[[S, 128], [B * HD * S, QH], [1, 128]])
            nc.sync.dma_start(out=dst, in_=qst[:])

            ptk = pstr_pool.tile([128, HD], F32)
            nc.tensor.transpose(ptk[:], k_ro[:], ident_r[:])
            nc.scalar.copy(kT_sb[:, tt, :], ptk[:])

    # ---------------- Phase 2: attention per (batch, head) ---------------
    with tc.tile_pool(name="at_q", bufs=2) as qpool, \
         tc.tile_pool(name="at_p", bufs=6) as ppool, \
         tc.tile_pool(name="at_pt", bufs=4) as ptpool, \
         tc.tile_pool(name="at_m", bufs=4) as mpool, \
         tc.tile_pool(name="at_st", bufs=24) as stat, \
         tc.tile_pool(name="at_o", bufs=3) as opool, \
         tc.tile_pool(name="at_ps_s", bufs=5, space="PSUM") as spsum, \
         tc.tile_pool(name="at_ps_pt", bufs=1, space="PSUM") as ptpsum, \
         tc.tile_pool(name="at_ps_o", bufs=2, space="PSUM") as opsum:

        for b in range(B):
            for h in range(QH):
                qTbh = qpool.tile([128, RT, 128], F32R)
                nc.sync.dma_start(out=qTbh[:], in_=qT_d[h, b])
                for qtg in range(4):
                    nkc = (qtg + 1) if causal else 4
                    ktmax = 4 * nkc
                    p_tiles = []
                    for qt in range(4):
                        r = qtg * 4 + qt
                        s_list, negmax = [], []
                        for kc in range(nkc):
                            sps = spsum.tile([128, 512], F32)
                            nc.tensor.matmul(
                                sps[:], qTbh[:, r, :],
                                kT_sb[:].rearrange("p t f -> p (t f)")
                                [:, (b * RT + kc * 4) * 128:
                                    (b * RT + kc * 4 + 4) * 128],
                                start=True, stop=True)
                            if mask_mode == "general" or (causal and kc == qtg):
                                mt = mpool.tile([128, 512], F32)
                                if causal:
                                    nc.sync.dma_start(
                                        out=mt[:],
                                        in_=mask_e.ap()[r * 128:(r + 1) * 128, :])
                                else:
                                    nc.sync.dma_start(
                                        out=mt[:],
                                        in_=mask_e.ap()[r * 128:(r + 1) * 128,
                                                        kc * 512:(kc + 1) * 512])
                                nc.vector.tensor_add(sps[:], sps[:], mt[:])
                            nm = stat.tile([128, 1], F32)
                            nc.vector.tensor_reduce(
                                nm[:], sps[:], axis=AX.X, op=OP.max, negate=True)
                            s_list.append(sps)
                            negmax.append(nm)
                        mfin = negmax[0]
                        for kc in range(1, nkc):
                            m2 = stat.tile([128, 1], F32)
                            nc.vector.tensor_tensor(m2[:], mfin[:], negmax[kc][:], op=OP.min)
                            mfin = m2
                        p_qt = ppool.tile([128, 4, 512], BF16)
                        stot = None
                        for kc in range(nkc):
                            sm = stat.tile([128, 1], F32)
                            nc.scalar.activation(
                                p_qt[:, kc, :], s_list[kc][:], AF.Exp,
                                bias=mfin[:], scale=1.0, accum_out=sm[:])
                            if stot is None:
                                stot = sm
                            else:
                                s2 = stat.tile([128, 1], F32)
                                nc.vector.tensor_add(s2[:], stot[:], sm[:])
                                stot = s2
                        rinv = stat.tile([128, 1], F32)
                        nc.vector.reciprocal(rinv[:], stot[:])
                        nc.vector.tensor_scalar_mul(
                            p_qt[:, 0:nkc, :], p_qt[:, 0:nkc, :], rinv[:])
                        p_tiles.append(p_qt)

                    otps = opsum.tile([128, 512], F32)
                    for kt in range(ktmax):
                        ptps = ptpsum.tile([128, 512], BF16)
                        for qt in range(4):
                            nc.tensor.transpose(
                                ptps[:, qt * 128:(qt + 1) * 128],
                                p_tiles[qt][:, kt // 4,
                                            (kt % 4) * 128:(kt % 4 + 1) * 128],
                                ident_b[:])
                        ptsb = ptpool.tile([128, 512], BF16)
                        nc.scalar.copy(ptsb[:], ptps[:])
                        nc.tensor.matmul(otps[:], v_sb[:, b * RT + kt, :],
                                         ptsb[:], start=(kt == 0),
                                         stop=(kt == ktmax - 1))
                    osb = opool.tile([128, 512], BF16)
                    nc.scalar.copy(osb[:], otps[:])
                    nc.sync.dma_start(
                        out=a2a_in[b * 4 + qtg, h * HD:(h + 1) * HD, :],
                        in_=osb[:])

    nc.gpsimd.collective_compute(
        "AllToAll", OP.bypass, replica_groups=[list(range(NCORES))],
        ins=[a2a_in.opt()], outs=[a2a_out.opt()])

    # ---------------- Phase 3: Wo GEMM on own token slice ----------------
    with tc.tile_pool(name="wo_of", bufs=1) as ofpool, \
         tc.tile_pool(name="wo_w", bufs=4) as wopool, \
         tc.tile_pool(name="wo_out", bufs=4) as outpool, \
         tc.tile_pool(name="wo_ps", bufs=1, space="PSUM") as wopsum:

        oTf = ofpool.tile([128, 32, TSLICE], BF16)
        for kk in range(32):
            d, fs = kk // 4, kk % 4
            nc.sync.dma_start(out=oTf[:, kk, :],
                              in_=a2a_out[d, fs * 128:(fs + 1) * 128, :])
        for dmc in range(8):
            pso = wopsum.tile([128, 4, 512], F32)
            for kk in range(32):
                wot = wopool.tile([128, 512], BF16)
                nc.sync.dma_start(
                    out=wot[:], in_=Wo_e.ap()[kk * 128:(kk + 1) * 128,
                                              dmc * 512:(dmc + 1) * 512])
                for m in range(4):
                    nc.tensor.matmul(pso[:, m, :],
                                     oTf[:, kk, m * 128:(m + 1) * 128],
                                     wot[:], start=(kk == 0), stop=(kk == 31))
            for m in range(4):
                osb = outpool.tile([128, 512], F32)
                nc.scalar.copy(osb[:], pso[:, m, :])
                nc.sync.dma_start(
                    out=out_e.ap()[m * 128:(m + 1) * 128,
                                   dmc * 512:(dmc + 1) * 512],
                    in_=osb[:])


_NC_CACHE = {}


def _get_nc(mask_mode):
    if mask_mode not in _NC_CACHE:
        _NC_CACHE[mask_mode] = _build(mask_mode)
    return _NC_CACHE[mask_mode]


def kernel(x, Wq, Wk, Wv, Wo, freqs_cos, freqs_sin, mask, start_pos=0,
           _want_trace=False):
    x = np.asarray(x, dtype=np.float32)
    mask = np.asarray(mask, dtype=np.float32)
    freqs_cos = np.asarray(freqs_cos, dtype=np.float32)
    freqs_sin = np.asarray(freqs_sin, dtype=np.float32)

    # mask classification on host (kernel math honors the mask in all modes)
    if not mask.any():
        mask_mode = "zeros"
    else:
        canon = np.where(np.tril(np.ones((S, S), bool)), 0.0,
                         np.float32(NEG_INF)).astype(np.float32)
        mask_mode = "causal" if np.array_equal(mask, canon) else "general"

    xT = np.ascontiguousarray(x.reshape(TOK, D).T)
    scale = np.float32(1.0 / math.sqrt(HD))
    cq = np.ascontiguousarray(freqs_cos * scale)
    sq = np.ascontiguousarray(freqs_sin * scale)
    Wo_bf = np.ascontiguousarray(np.asarray(Wo, np.float32)
                                 .astype(ml_dtypes.bfloat16))
    if mask_mode == "causal":
        maskd = np.empty((S, 512), np.float32)
        for r in range(RT):
            c0 = (r // 4) * 512
            maskd[r * 128:(r + 1) * 128] = mask[r * 128:(r + 1) * 128,
                                                c0:c0 + 512]

    in_maps = []
    for c in range(NCORES):
        m = {
            "xT": xT,
            "Wq": np.ascontiguousarray(Wq[:, c * QH * HD:(c + 1) * QH * HD]),
            "Wkv": np.ascontiguousarray(
                np.concatenate([Wk[:, c * HD:(c + 1) * HD],
                                Wv[:, c * HD:(c + 1) * HD]], axis=1)),
            "Wo": Wo_bf,
            "cq": cq, "sq": sq,
            "ck": freqs_cos, "sk": freqs_sin,
        }
        if mask_mode == "causal":
            m["maskd"] = maskd
        elif mask_mode == "general":
            m["mask"] = mask
        in_maps.append(m)

    nc = _get_nc(mask_mode)
    res = run_bass_kernel_spmd(nc, in_maps, list(range(NCORES)),
                               trace=_want_trace)
    out = np.concatenate([res.results[c]["out"] for c in range(NCORES)],
                         axis=0).reshape(B, S, D)
    if _want_trace:
        return out, res
    return out


if __name__ == "__main__":
    rng = np.random.default_rng(0)
    x = rng.standard_normal((B, S, D), dtype=np.float32) * 0.1
    Wq = rng.standard_normal((D, NH * HD), dtype=np.float32) * 0.02
    Wk = rng.standard_normal((D, NKV * HD), dtype=np.float32) * 0.02
    Wv = rng.standard_normal((D, NKV * HD), dtype=np.float32) * 0.02
    Wo = rng.standard_normal((NH * HD, D), dtype=np.float32) * 0.02
    fc = rng.standard_normal((S, 64), dtype=np.float32)
    fs = rng.standard_normal((S, 64), dtype=np.float32)
    mask = np.where(np.tril(np.ones((S, S), bool)), 0.0,
                    np.float32(NEG_INF)).astype(np.float32)
    out = kernel(x, Wq, Wk, Wv, Wo, fc, fs, mask, 0)
    print("out", out.shape, out.dtype, np.abs(out).mean())
